# revision 25
# baseline (speedup 1.0000x reference)
"""Block-diagonal compress kernel: out = blockdiag(A) @ W @ blockdiag(B).

Shapes (full): W [8192, 8192] f32, A_blocks [128, 64, 64], B_blocks [128, 64, 64].
Sharding: row-shard W / A over 8 cores (1024 rows = 16 A-blocks each);
B replicated.  Each core computes outT = (A_bd @ W_shard @ B_bd)^T and the
host transposes each shard back on gather.

The rel-err gate is 2e-2 and the data is N(0,1), so a single bf16
representation of W / A / B / T / out keeps the total error ~4e-3 while
halving HBM traffic on both directions vs an f32/hi-lo scheme: 16 MB W
read + 16 MB outT write per core (~95 us at the ~358 GB/s/core HBM
limit).

Per-core dataflow (per 1024-column group g, software-pipelined one group
deep so PE never waits on fresh PSUM->SBUF copies):
  load   W(g):  two 1 MB half DMAs on the SP HWDGE queue into wt halves
                [128, 4, 1024].  W is host-retiled [G, 128, R, 1024] so
                each partition line is 8 KB contiguous (big descriptors);
                halves + bufs=6 prefetch W three groups deep, hiding the
                ~8 us DMA behind two groups of compute.
  step 1 T(g):  (A_bd @ W)^T chunk-wise with W as the matmul *stationary*
                operand: matmul(lhsT=W[128 rows, 128 cols], rhs=A^T-packed
                pair-blockdiag) -> psum [128 cols, 128 rows].  This absorbs
                the transpose the chained matmul otherwise needs.  DVE
                casts psum -> tg bf16 flat (strides moved to the step-2
                rhs AP instead of the DVE write).
  step 2 (g-1): outT[chunk] = matmul(lhsT=blockdiag(B_j0,B_j1) bf16,
                rhs=T rows-chunk [4 r-slabs x 128] strided), free dim 512.
                ACT copies psum -> og (bf16), then one 2 MB store per
                group on the ACT HWDGE queue ([G, 128, 8*1024] host
                layout -> 16 KB descriptors); the last store is split in
                half to shorten the drain tail.
Engine budget per group: DMA ~12.5 us (bottleneck), DVE 8 casts ~11.7 us,
ACT 8 copies + trigger ~11.5 us, PE ~9 us.
"""

import bass_rust
import numpy as np

import concourse.bass as bass
import concourse.mybir as mybir
from concourse.bass_utils import run_bass_kernel_spmd
from concourse.tile import TileContext

F32 = mybir.dt.float32
BF16 = mybir.dt.bfloat16

N_CORES = 8
D = 8192
BLK = 64
ROWS_PC = D // N_CORES  # 1024 rows of W / out per core

_HOIST_OPCODES = {"Matmult", "DMACopy", "TensorCopy", "Memset", "Activation", "Drain"}


def _hoist_excess_matmul_waits(nc: bass.Bass, max_waits: int = 1) -> None:
    """walrus's codegen for several instruction structs (fused-LDWEIGHTS
    matmul, DMA_DIRECT2D, ...) has few sync-wait slots ("Too many sync wait
    commands"). Move excess semaphore waits off such instructions into
    standalone EventSemaphore instructions right before them on the same
    engine queue — the sequencer executes those in order, so the instruction
    still starts only after all waits pass."""
    ctr = 0
    for fnc in nc.m.functions:
        for bb in fnc.blocks:
            new = []
            for ins in bb.instructions:
                si = ins.sync_info if ins.opcode in _HOIST_OPCODES else None
                if si is not None and len(si.on_wait) > max_waits:
                    waits = list(si.on_wait)
                    for w in waits[:-max_waits]:
                        evs = mybir.InstEventSemaphore(
                            name=f"mmwaithoist-{ctr}", ins=[], outs=[]
                        )
                        ctr += 1
                        evs.engine = ins.engine
                        evs.sync_info = bass_rust.SyncInfo(on_wait=[w], on_update=[])
                        new.append(evs)
                    ins.sync_info.on_wait = waits[-max_waits:]
                new.append(ins)
            bb.instructions[:] = new


def build_nc(rows_pc: int = ROWS_PC, d: int = D, hoist: bool = True) -> bass.Bass:
    """One-core SPMD program. rows_pc/d scaled down only for sim tests.
    hoist=False keeps waits on the original instructions (CoreSim's race
    detector wants every instruction to carry its own updates; the hoisted
    variant is for walrus, whose ISA structs have too few wait slots)."""
    R = rows_pc // 128  # 128-row slabs per core (= A-block pairs)
    G = d // 1024       # 1024-wide column groups
    n2 = rows_pc // 512  # 512-row chunks for step-2 free dim
    RH = R // 2          # r-slabs per W half-tile

    nc = bass.Bass()
    # W retiled so each partition line of a half-load is RH*1024 contiguous.
    w_ext = nc.declare_dram_parameter("wh", [G, 2, 128, RH * 1024], BF16,
                                      isOutput=False)
    a_ext = nc.declare_dram_parameter("ah", [128, R * 128], BF16, isOutput=False)
    b_ext = nc.declare_dram_parameter("bpack", [128, d], BF16, isOutput=False)
    # outT stored [g, p, cc*rows]; host untiles back to [d, rows_pc].
    ot_ext = nc.declare_dram_parameter(
        "outt", [G, 128, 8 * rows_pc], BF16, isOutput=True
    )

    with TileContext(nc) as tc:
        with (
            tc.tile_pool(name="const", bufs=1) as cpool,
            tc.tile_pool(name="wp", bufs=6) as wpool,
            tc.tile_pool(name="tg", bufs=4) as tpool,
            tc.tile_pool(name="op", bufs=3) as opool,
            tc.tile_pool(name="p1", bufs=2, space="PSUM") as p1pool,
            tc.tile_pool(name="p2", bufs=2, space="PSUM") as p2pool,
        ):
            # A first (gates the first matmul) on the ACT HWDGE queue which
            # is otherwise idle until the first cast; bpack is emitted AFTER
            # the early W loads below (FIFO order) — it isn't needed until
            # the first chunk2 at ~group 2.
            ah = cpool.tile([128, R * 128], BF16)
            nc.scalar.dma_start(out=ah[:], in_=a_ext[:])

            def load_w(g, parts=2, engines=None):
                """Load W(g) as `parts` tiles.  The pipeline fill is W-DMA
                bound (stores idle, single ring at cold-start rate), so the
                early groups split their parts across BOTH HWDGE rings via
                `engines`.  Returns (tiles, slabs per part)."""
                tiles = []
                rpp = R // parts
                for h in range(parts):
                    wt = wpool.tile([128, rpp * 1024], BF16, name="wt")
                    eng = engines[h] if engines else nc.sync
                    eng.dma_start(
                        out=wt[:],
                        in_=w_ext[g, h // (parts // 2), :,
                                  (h % (parts // 2)) * rpp * 1024 :
                                  (h % (parts // 2) + 1) * rpp * 1024],
                    )
                    tiles.append(wt)
                return tiles, rpp

            def slab1(g, wparts, tg, r, act=False):
                """One 128-row slab of T(g) = (A_bd @ W cols g)^T: 8 matmuls
                into psum + one psum->tg cast (flat layout
                tg[p, r*1024 + cc*128 + n], same order p1 is produced in).
                act=True drains the cast on ACT instead of DVE — used for
                groups 0-1 where ACT has no chunk2 copies yet, doubling cast
                throughput while the pipeline fills."""
                tiles, rpp = wparts
                wt = tiles[r // rpp]
                rr = r % rpp
                p1 = p1pool.tile([128, 1024], F32)
                for cc in range(8):
                    cs = slice(cc * 128, (cc + 1) * 128)
                    nc.tensor.matmul(
                        p1[:, cs],
                        lhsT=wt[:, rr * 1024 + cc * 128 : rr * 1024 + (cc + 1) * 128],
                        rhs=ah[:, r * 128 : (r + 1) * 128],
                        start=True, stop=True,
                    )
                if act:
                    nc.scalar.copy(tg[:, r * 1024 : (r + 1) * 1024], p1[:])
                else:
                    nc.vector.tensor_copy(tg[:, r * 1024 : (r + 1) * 1024], p1[:])

            def chunk2(g, tg, og, cc, last):
                """outT rows (8g+cc)*128..+128 = T(g) chunk scaled by B block
                pair j2: 2 matmuls (free dim 512) + one psum->og copy."""
                tgv = tg[:].rearrange("p (r cc n) -> p cc r n", r=R, cc=8)
                ogv = og[:].rearrange("p (cc w) -> p cc w", cc=8)
                j2 = 8 * g + cc
                # Only the FINAL group's chunks (g == G-1, emitted after all
                # slabs) may touch DVE or step-1's psum pool: anything queued
                # on DVE before that sits statically behind the remaining
                # slab casts and drags the whole chain there.  The
                # second-to-last group drains purely on ACT/p2pool,
                # concurrently with the final group's DVE chain.
                dve = last and cc % 2 == 0
                if last and cc % 2 == 0:
                    p2 = p1pool.tile([128, rows_pc], F32, name="p1")
                else:
                    p2 = p2pool.tile([128, rows_pc], F32, name="p2")
                lb = bp[:, j2 * 128 : (j2 + 1) * 128]
                rh = R // n2  # r-slabs per 512-row chunk
                for s in range(n2):
                    nc.tensor.matmul(
                        p2[:, s * 512 : (s + 1) * 512],
                        lhsT=lb,
                        rhs=tgv[:, cc, s * rh : (s + 1) * rh, :],
                        start=True, stop=True,
                    )
                if dve:
                    nc.vector.tensor_copy(ogv[:, cc, :], p2[:])
                else:
                    nc.scalar.copy(ogv[:, cc, :], p2[:])

            def store(g, og):
                # Stores ride the gpsimd SWDGE: both HWDGE queues are
                # loaded (SP: W prefetch, ACT: copy chain is the critical
                # engine and each DMA trigger costs it ~0.7 us — ACT falls
                # ~0.8 us/group behind DVE and the deficit drains as a
                # serial tail).  gpsimd is otherwise idle.
                if g == G - 1:
                    # quarter the final store so the drain tracks the copies
                    for q in range(4):
                        qs_ = slice(q * 2 * rows_pc, (q + 1) * 2 * rows_pc)
                        nc.gpsimd.dma_start(out=ot_ext[g, :, qs_], in_=og[:, qs_])
                elif g == G - 2:
                    half = 4 * rows_pc
                    nc.gpsimd.dma_start(out=ot_ext[g, :, :half], in_=og[:, :half])
                    nc.gpsimd.dma_start(out=ot_ext[g, :, half:], in_=og[:, half:])
                else:
                    nc.gpsimd.dma_start(out=ot_ext[g], in_=og[:])

            # Software pipeline: step-2 runs TWO groups behind step-1.  The
            # Tile scheduler is a timing-sim list scheduler — step-2 of g-1
            # only becomes "ready" after the last cast of group g-1, so it
            # loses its PE slot and bunches into a burst that stalls the
            # copy engines ~6 us at every group boundary.  tg(g-2) is fully
            # written a whole group earlier, so its chunk2 work is always
            # ready and fills PE gaps between cast-gated step-1 slabs.
            wq = []
            if G >= 3:
                wq.append(load_w(0, parts=4,
                                 engines=[nc.sync, nc.scalar, nc.sync, nc.scalar]))
                wq.append(load_w(1, parts=2, engines=[nc.sync, nc.scalar]))
                wq.append(load_w(2, parts=2, engines=[nc.sync, nc.scalar]))
            else:
                wq = [load_w(g) for g in range(G)]
            bp = cpool.tile([128, d], BF16)
            nc.scalar.dma_start(out=bp[:], in_=b_ext[:])
            tgs: dict = {}
            ogs: dict = {}
            for g in range(G + 2):
                if g < G:
                    tgs[g] = tpool.tile([128, 8 * rows_pc], BF16, name="tg")
                if g >= 2:
                    ogs[g - 2] = opool.tile([128, 8 * rows_pc], BF16, name="og")
                for r in range(R):
                    if g < G:
                        # groups 0-1: ACT has no chunk2 copies yet — split
                        # the casts across both engines to fill the pipe 2x
                        # faster
                        slab1(g, wq[0], tgs[g], r, act=(g < 2 and r % 2 == 1))
                    if g >= 2 and r < 8:
                        chunk2(g - 2, tgs[g - 2], ogs[g - 2], cc=r, last=g > G)
                if g >= 2:
                    for cc in range(R, 8):
                        chunk2(g - 2, tgs[g - 2], ogs[g - 2], cc, last=g > G)
                if g < G:
                    wq.pop(0)
                    if g + 3 < G:
                        wq.append(load_w(g + 3))
                if g >= 2:
                    store(g - 2, ogs[g - 2])
                    del tgs[g - 2], ogs[g - 2]
    if hoist:
        _hoist_excess_matmul_waits(nc)
    return nc


def pack_at(a_blocks: np.ndarray) -> np.ndarray:
    """[2R, 64, 64] A blocks -> bf16 [128, R*128] with
    out[64*b + k, 128*r + 64*b + n] = A[2r+b][n, k] (transposed, pair-blockdiag)."""
    import ml_dtypes

    nb = a_blocks.shape[0]
    R = nb // 2
    out = np.zeros((128, R * 128), np.float32)
    at = a_blocks.transpose(0, 2, 1)
    out[0:64].reshape(64, R, 2, 64)[:, :, 0, :] = at[0::2].transpose(1, 0, 2)
    out[64:128].reshape(64, R, 2, 64)[:, :, 1, :] = at[1::2].transpose(1, 0, 2)
    return out.astype(ml_dtypes.bfloat16)


def pack_b(b_blocks: np.ndarray) -> np.ndarray:
    """[2J, 64, 64] B blocks -> bf16 [128, J*128] with
    out[64*b + k, 128*j + 64*b + n] = B[2j+b][k, n] (pair-blockdiag, untransposed)."""
    import ml_dtypes

    nb = b_blocks.shape[0]
    J = nb // 2
    out = np.zeros((128, J * 128), np.float32)
    out[0:64].reshape(64, J, 2, 64)[:, :, 0, :] = b_blocks[0::2].transpose(1, 0, 2)
    out[64:128].reshape(64, J, 2, 64)[:, :, 1, :] = b_blocks[1::2].transpose(1, 0, 2)
    return out.astype(ml_dtypes.bfloat16)


def pack_w(w_shard: np.ndarray):
    """[rows_pc, d] -> bf16 [G, 2, 128, (R/2)*1024]: per column group g and
    half h, partition p holds W[4h*128 .. 4h*128+512 rows at stride 128 ->
    rows (4h+rr)*128+p][g*1024 : (g+1)*1024] as one contiguous 4 KB line."""
    import ml_dtypes

    rows_pc, d = w_shard.shape
    R, G = rows_pc // 128, d // 1024
    # [R, 128, G, 1024] -> [G, R, 128, 1024] -> [G, 2, RH, 128, 1024]
    wt = w_shard.reshape(R, 128, G, 1024).transpose(2, 0, 1, 3)
    wt = wt.reshape(G, 2, R // 2, 128, 1024).transpose(0, 1, 3, 2, 4)
    wt = wt.reshape(G, 2, 128, (R // 2) * 1024)
    return np.ascontiguousarray(wt).astype(ml_dtypes.bfloat16)


_NC_CACHE: dict = {}


def run(W, A_blocks, B_blocks, trace: bool = False, trace_cores=None):
    W = np.asarray(W, dtype=np.float32)
    A_blocks = np.asarray(A_blocks, dtype=np.float32)
    B_blocks = np.asarray(B_blocks, dtype=np.float32)
    assert W.shape == (D, D) and A_blocks.shape == (D // BLK, BLK, BLK)

    if "nc" not in _NC_CACHE:
        _NC_CACHE["nc"] = build_nc()
    nc = _NC_CACHE["nc"]

    bp = pack_b(B_blocks)
    in_maps = []
    for c in range(N_CORES):
        wh = pack_w(W[ROWS_PC * c : ROWS_PC * (c + 1)])
        ah = pack_at(A_blocks[16 * c : 16 * (c + 1)])
        in_maps.append({"wh": wh, "ah": ah, "bpack": bp})
    res = run_bass_kernel_spmd(
        nc, in_maps, core_ids=list(range(N_CORES)), trace=trace, trace_cores=trace_cores
    )
    out = np.empty((D, D), np.float32)
    for c in range(N_CORES):
        # [G, 128, 8, 1024] -> outT [G, 8, 128, 1024] -> [d, rows_pc] -> T
        ot = np.asarray(res.results[c]["outt"]).reshape(8, 128, 8, ROWS_PC)
        ot = ot.transpose(0, 2, 1, 3).reshape(D, ROWS_PC).astype(np.float32)
        out[ROWS_PC * c : ROWS_PC * (c + 1), :] = ot.T
    return out, res


def kernel(W, A_blocks, B_blocks):
    out, _ = run(W, A_blocks, B_blocks, trace=False)
    return out
